# Initial kernel scaffold
#
"""CE + CJS loss kernel for Trainium2, data-parallel over 8 NeuronCores.

Math (reference):
    logp = log_softmax(pred_logit, axis=1)          # x - lse_i
    ce   = -mean_i( sum_j gt*logp )
    p    = softmax(pred_logit)
    m    = 0.5*(gt + p + EPS)
    contrib = gt*ln(gt) + p*logp - (gt+p)*ln(m)     # per element
    cjs  = 0.5 * sum_ij w_j * contrib_ij / B,  w_j = C - j
    loss = ce + 0.5*cjs

Kernel decomposition (everything reduces to grand totals):
    e1 = gt*ln(gt);  e2 = u*q;  e3 = gt*x'
    with u = gt+p, x' = x-lse, q = x'-ln(m)
    contrib = e1 + e2 - e3
    CE total  = sum_ij e3
    E1 (cjs)  = sum_j w_j * colsum(e1+e2-e3)_j
Column sums over the batch are computed by TensorE (ones-vector matmuls
accumulated in PSUM); the w-weighted dot and final assembly happen on the
host in float64. Each core handles a 512-row shard of the batch and
returns two [16,512] column-sum planes; no collectives needed.
"""
import numpy as np

import concourse.bass as bass
import concourse.tile as tile
from concourse import mybir
from concourse.bass_utils import run_bass_kernel_spmd
from concourse.vector_clock import ScopedClock

B, C = 4096, 8192
N_CORES = 8
ROWS = B // N_CORES          # 512 rows per core
N_BLK = ROWS // 128          # 4 partition blocks
F2 = 2048                    # sweep-2 chunk width
N_CHUNK = C // F2            # 8 chunks per block
N_SLICE = C // 512           # 16 matmul column slices
EPS = 1e-8
# All log-magnitude tensors (lngt, logm, x-lse) sit near -9.2 where bf16's
# ulp is 0.0625; shifting them by +K before the bf16 round shrinks the
# quantization bias ~8x. The shift cancels exactly in lngt-logm and
# (x-lse)-logm; the CE plane carries it explicitly and the host removes it.
K_SHIFT = 9.2
# scalar_tensor_tensor has no 2x uop on cayman (1x: ~2x slower than the
# explicit tensor_scalar 4x + tensor_tensor 2x pair) — measured on HW.
USE_STT = False
GT16_ON_ACT = True  # f32->bf16 cast of gt on ScalarE instead of VectorE
XP_ON_ACT = False   # xp = x - lse on ScalarE (Identity w/ bias) vs VectorE
_EK = float(np.float32(np.exp(K_SHIFT)))          # scale for Ln(gt), Ln(m)
_EMK = float(np.float32(np.exp(-K_SHIFT)))        # scale for lse
K_LSE = -float(np.log(np.float64(_EMK)))          # effective shift on x-lse

f32 = mybir.dt.float32
bf16 = mybir.dt.bfloat16
AF = mybir.ActivationFunctionType
ALU = mybir.AluOpType


def _patched_drain_and_barrier(self, tick_clock, wait_clock):
    # Walrus CoreV3 codegen allows only ONE sync-wait command on a
    # Drain/NoOp (NO_STRUCT ctrl). The stock Tile tail drain carries one
    # wait per pending engine clock and fails to compile. Split the waits
    # across single-wait SP nops; SP executes in program order, so the
    # drain still orders after everything.
    nc = self.nc
    probe = nc.sync.nop().ins
    wait_clock.add_sem_waits(probe, ScopedClock({None: tick_clock.global_clock}))
    waits = list(probe.sync_info.on_wait) if probe.sync_info else []
    probe.sync_info = mybir.SyncInfo(on_wait=waits[:1], on_update=[])
    for w in waits[1:]:
        extra = nc.sync.nop().ins
        extra.sync_info = mybir.SyncInfo(on_wait=[w], on_update=[])
    nc.sync.drain()
    nc.all_engine_barrier()
    assert self.sems is not None
    popped = nc._tile_sem_poison_stack.pop()
    assert popped is self._sem_poison
    nc.clear_and_free_semaphores(list(self.sems.allocated().values()))
    nc.all_engine_barrier()


tile.TileContext._drain_and_barrier = _patched_drain_and_barrier


def _split_excess_waits(nc: bass.Bass, max_waits: int = 1):
    # Same walrus limitation, general form: cap sync waits per instruction,
    # hoisting the excess onto same-engine NOPs inserted just before (the
    # engine executes its stream in order, so semantics are unchanged).
    for bb in nc.main_func.blocks:
        insts = list(bb.instructions)
        out, changed = [], False
        for ins in insts:
            si = ins.sync_info
            waits = list(si.on_wait) if (si is not None and si.on_wait) else []
            if len(waits) > max_waits:
                ups = list(si.on_update) if si.on_update else []
                for w in waits[:-max_waits]:
                    nop = mybir.InstNoOp(
                        name=nc.get_next_instruction_name(), ins=[], outs=[])
                    nop.engine = ins.engine
                    nop.sync_info = mybir.SyncInfo(on_wait=[w], on_update=[])
                    nc.register_instruction(nop)
                    out.append(nop)
                ins.sync_info = mybir.SyncInfo(
                    on_wait=waits[-max_waits:], on_update=ups)
                changed = True
            out.append(ins)
        if changed:
            bb.instructions = out


def build_nc(bench_iters: int = 0) -> bass.Bass:
    # bench_iters>0 wraps the compute body in a HW For_i loop so one
    # dispatch runs it N times (timing two N values cancels dispatch cost).
    nc = bass.Bass()
    x_dram = nc.declare_dram_parameter("pred_logit", [ROWS, C], f32, isOutput=False)
    gt_dram = nc.declare_dram_parameter("gt", [ROWS, C], f32, isOutput=False)
    out_dram = nc.declare_dram_parameter("partials", [N_SLICE, 512], f32, isOutput=True)
    ce_dram = nc.declare_dram_parameter("ce_part", [1, 512], f32, isOutput=True)

    from contextlib import ExitStack
    with tile.TileContext(nc) as tc, ExitStack() as es:
        consts = es.enter_context(tc.tile_pool(name="consts", bufs=1))
        xpool = es.enter_context(tc.tile_pool(name="xpool", bufs=2))
        tpool = es.enter_context(tc.tile_pool(name="tpool", bufs=2))
        rowp = es.enter_context(tc.tile_pool(name="rowp", bufs=2))
        ck = es.enter_context(tc.tile_pool(name="ck", bufs=2))
        psum = es.enter_context(tc.tile_pool(name="psum", bufs=1, space="PSUM"))

        ones = consts.tile([128, 1], bf16)
        nc.vector.memset(ones, 1.0)
        neg_ones = consts.tile([128, 1], bf16)
        nc.vector.memset(neg_ones, -1.0)
        eps_half = consts.tile([128, 1], f32)
        nc.vector.memset(eps_half, 0.5 * EPS * _EK)

        # Column-sum accumulators for contrib = e1+e2-e3: one [1,512] row
        # per 512-column slice. PE output base partition must be 0/32/64,
        # so pack 3 slices per PSUM bank at those bases.
        banks = [psum.tile([128, 512], f32, name=f"csbank{i}", tag=f"csbank{i}")
                 for i in range((N_SLICE + 2) // 3)]
        def cs_ap(m):
            bank, base = banks[m // 3], 32 * (m % 3)
            return bank[base:base + 1, :]
        # CE accumulator: column sums of e3 from ALL slices superimposed
        # into one 512-wide row; the host sums the 512 values.
        ce_psum = psum.tile([1, 512], f32)

        N_XSUB = 4
        XS = C // N_XSUB

        def emit_body():
          for b in range(N_BLK):
            r0 = b * 128
            # split the x load + exp pass into sub-chunks so ACT can start
            # as soon as the first piece lands (instead of stalling ~12us
            # behind one monolithic 4MB DMA)
            xb = xpool.tile([128, C], f32, tag="x")
            tb = tpool.tile([128, C], bf16, tag="t")
            s4 = rowp.tile([128, N_XSUB], f32, tag="s4")
            for ix in range(N_XSUB):
                xsl = slice(ix * XS, (ix + 1) * XS)
                nc.sync.dma_start(out=xb[:, xsl], in_=x_dram[r0:r0 + 128, xsl])
                nc.scalar.activation(
                    out=tb[:, xsl], in_=xb[:, xsl], func=AF.Exp,
                    accum_out=s4[:, ix:ix + 1],
                )
            s = rowp.tile([128, 1], f32, tag="s")
            nc.vector.tensor_reduce(
                out=s[:], in_=s4[:], op=ALU.add, axis=mybir.AxisListType.X,
            )

            recip = rowp.tile([128, 1], f32, tag="recip")
            nc.vector.reciprocal(out=recip[:], in_=s[:])
            # lse - K  (so xp = x - lse + K comes out of one tensor_scalar)
            lse = rowp.tile([128, 1], f32, tag="lse")
            nc.scalar.activation(out=lse[:], in_=s[:], func=AF.Ln, scale=_EMK)
            neg_lse = rowp.tile([128, 1], f32, tag="neg_lse")
            if XP_ON_ACT:
                nc.scalar.mul(neg_lse[:], lse[:], -1.0)

            for c in range(N_CHUNK):
                j0 = c * F2
                gtc = ck.tile([128, F2], f32, tag="gt")
                nc.sync.dma_start(out=gtc[:], in_=gt_dram[r0:r0 + 128, j0:j0 + F2])

                gt16 = ck.tile([128, F2], bf16, tag="gt16")
                if GT16_ON_ACT:
                    nc.scalar.copy(out=gt16[:], in_=gtc[:])
                else:
                    nc.vector.tensor_copy(out=gt16[:], in_=gtc[:])
                lngt = ck.tile([128, F2], bf16, tag="lngt")
                nc.scalar.activation(out=lngt[:], in_=gtc[:], func=AF.Ln, scale=_EK)

                xp = ck.tile([128, F2], bf16, tag="xp")
                if XP_ON_ACT:
                    nc.scalar.activation(
                        out=xp[:], in_=xb[:, j0:j0 + F2], func=AF.Identity,
                        bias=neg_lse[:],
                    )
                else:
                    nc.vector.tensor_scalar(
                        out=xp[:], in0=xb[:, j0:j0 + F2],
                        scalar1=lse[:], scalar2=None, op0=ALU.subtract,
                    )
                # u = t*recip + gt = p + gt
                u = ck.tile([128, F2], bf16, tag="u")
                if USE_STT:
                    nc.vector.scalar_tensor_tensor(
                        out=u[:], in0=tb[:, j0:j0 + F2], scalar=recip[:],
                        in1=gt16[:], op0=ALU.mult, op1=ALU.add,
                    )
                else:
                    p = ck.tile([128, F2], bf16, tag="p", bufs=1)
                    nc.vector.tensor_scalar(
                        out=p[:], in0=tb[:, j0:j0 + F2],
                        scalar1=recip[:], scalar2=None, op0=ALU.mult,
                    )
                    nc.vector.tensor_tensor(out=u[:], in0=gt16[:], in1=p[:], op=ALU.add)
                logm = ck.tile([128, F2], bf16, tag="logm")
                nc.scalar.activation(
                    out=logm[:], in_=u[:], func=AF.Ln, scale=0.5 * _EK,
                    bias=eps_half[:],
                )
                q = ck.tile([128, F2], bf16, tag="q")
                nc.vector.tensor_tensor(out=q[:], in0=xp[:], in1=logm[:], op=ALU.subtract)
                r = ck.tile([128, F2], bf16, tag="r")
                nc.vector.tensor_tensor(out=r[:], in0=lngt[:], in1=logm[:], op=ALU.subtract)

                # contrib = gt*(lngt-logm) + p*(xp-logm)  (shifts cancel)
                e1 = ck.tile([128, F2], bf16, tag="e1", bufs=1)
                nc.vector.tensor_tensor(out=e1[:], in0=gt16[:], in1=r[:], op=ALU.mult)
                # e2 = (t*recip)*q = p*q
                e2 = ck.tile([128, F2], bf16, tag="e2", bufs=1)
                if USE_STT:
                    nc.vector.scalar_tensor_tensor(
                        out=e2[:], in0=tb[:, j0:j0 + F2], scalar=recip[:],
                        in1=q[:], op0=ALU.mult, op1=ALU.mult,
                    )
                else:
                    nc.vector.tensor_tensor(out=e2[:], in0=p[:], in1=q[:], op=ALU.mult)
                e3 = ck.tile([128, F2], bf16, tag="e3", bufs=1)
                nc.vector.tensor_tensor(out=e3[:], in0=gt16[:], in1=xp[:], op=ALU.mult)

                for k in range(F2 // 512):
                    m = (j0 + k * 512) // 512
                    sl = slice(k * 512, (k + 1) * 512)
                    nc.tensor.matmul(
                        cs_ap(m), ones[:], e1[:, sl],
                        start=(b == 0), stop=False,
                    )
                    nc.tensor.matmul(
                        cs_ap(m), ones[:], e2[:, sl],
                        start=False, stop=(b == N_BLK - 1),
                    )
                    nc.tensor.matmul(
                        ce_psum[:], ones[:], e3[:, sl],
                        start=(b == 0 and m == 0),
                        stop=(b == N_BLK - 1 and m == N_SLICE - 1),
                    )

        if bench_iters > 1:
            with tc.For_i(0, bench_iters, 1):
                emit_body()
        else:
            emit_body()

        # PSUM is not DMA-readable: bounce through SBUF via ScalarE.
        sb_banks = [consts.tile([128, 512], f32, name=f"sb_cs{i}", tag=f"sb_cs{i}")
                    for i in range(len(banks))]
        for i, bank in enumerate(banks):
            nc.scalar.copy(out=sb_banks[i][:], in_=bank[:])
        sb_ce = consts.tile([1, 512], f32)
        nc.scalar.copy(out=sb_ce[:], in_=ce_psum[:])
        for m in range(N_SLICE):
            bank, base = sb_banks[m // 3], 32 * (m % 3)
            nc.sync.dma_start(out=out_dram[m:m + 1, :], in_=bank[base:base + 1, :])
        nc.sync.dma_start(out=ce_dram[:], in_=sb_ce[:])

    _split_excess_waits(nc)
    return nc


_NC_CACHE = None


def kernel(pred_logit: np.ndarray, gt: np.ndarray) -> np.ndarray:
    global _NC_CACHE
    if _NC_CACHE is None:
        _NC_CACHE = build_nc()
    nc = _NC_CACHE

    pred_logit = np.ascontiguousarray(pred_logit, dtype=np.float32)
    gt = np.ascontiguousarray(gt, dtype=np.float32)
    in_maps = [
        {
            "pred_logit": pred_logit[c * ROWS:(c + 1) * ROWS],
            "gt": gt[c * ROWS:(c + 1) * ROWS],
        }
        for c in range(N_CORES)
    ]
    res = run_bass_kernel_spmd(nc, in_maps, list(range(N_CORES)))

    w = (C - np.arange(C)).astype(np.float64)
    e1_total = 0.0   # sum_ij w_j * contrib
    ce_total = 0.0   # sum_ij gt * logp
    for r in res.results:
        cs = r["partials"].astype(np.float64).reshape(C)
        e1_total += np.dot(w, cs)
        # ce plane accumulated gt*(x - lse + K); remove the K shift
        # (rows of gt sum to 1, so sum_ij K*gt = K*ROWS per core)
        ce_total += float(r["ce_part"].astype(np.float64).sum()) - K_LSE * ROWS
    loss = -ce_total / B + 0.25 * e1_total / B
    return np.array(loss, dtype=np.float32)



# revision 4
# speedup vs baseline: 1.2431x; 1.2431x over previous
"""CE + CJS loss kernel for Trainium2, data-parallel over 8 NeuronCores.

Math (reference):
    logp = log_softmax(pred_logit, axis=1)          # x - lse_i
    ce   = -mean_i( sum_j gt*logp )
    p    = softmax(pred_logit)
    m    = 0.5*(gt + p + EPS)
    contrib = gt*ln(gt) + p*logp - (gt+p)*ln(m)     # per element
    cjs  = 0.5 * sum_ij w_j * contrib_ij / B,  w_j = C - j
    loss = ce + 0.5*cjs

Kernel decomposition (everything reduces to grand totals):
    e1 = gt*ln(gt);  e2 = u*q;  e3 = gt*x'
    with u = gt+p, x' = x-lse, q = x'-ln(m)
    contrib = e1 + e2 - e3
    CE total  = sum_ij e3
    E1 (cjs)  = sum_j w_j * colsum(e1+e2-e3)_j
Column sums over the batch are computed by TensorE (ones-vector matmuls
accumulated in PSUM); the w-weighted dot and final assembly happen on the
host in float64. Each core handles a 512-row shard of the batch and
returns two [16,512] column-sum planes; no collectives needed.
"""
import os

import numpy as np

import concourse.bass as bass
import concourse.tile as tile
from concourse import mybir
from concourse.bass_utils import run_bass_kernel_spmd
from concourse.vector_clock import ScopedClock

B, C = 4096, 8192
N_CORES = 8
ROWS = B // N_CORES          # 512 rows per core
N_BLK = ROWS // 128          # 4 partition blocks
F2 = 2048                    # sweep-2 chunk width
N_CHUNK = C // F2            # 8 chunks per block
N_SLICE = C // 512           # 16 matmul column slices
EPS = 1e-8
# All log-magnitude tensors (lngt, logm, x-lse) sit near -9.2 where bf16's
# ulp is 0.0625; shifting them by +K before the bf16 round shrinks the
# quantization bias ~8x. The shift cancels exactly in lngt-logm and
# (x-lse)-logm; the CE plane carries it explicitly and the host removes it.
K_SHIFT = 9.2
# scalar_tensor_tensor has no 2x uop on cayman (1x: ~2x slower than the
# explicit tensor_scalar 4x + tensor_tensor 2x pair) — measured on HW.
USE_STT = False
GT16_ON_ACT = True  # f32->bf16 cast of gt on ScalarE instead of VectorE
XP_ON_ACT = False   # xp = x - lse on ScalarE (Identity w/ bias) vs VectorE
_EK = float(np.float32(np.exp(K_SHIFT)))          # scale for Ln(gt), Ln(m)
_EMK = float(np.float32(np.exp(-K_SHIFT)))        # scale for lse
K_LSE = -float(np.log(np.float64(_EMK)))          # effective shift on x-lse

f32 = mybir.dt.float32
bf16 = mybir.dt.bfloat16
AF = mybir.ActivationFunctionType
ALU = mybir.AluOpType


def _patched_drain_and_barrier(self, tick_clock, wait_clock):
    # Walrus CoreV3 codegen allows only ONE sync-wait command on a
    # Drain/NoOp (NO_STRUCT ctrl). The stock Tile tail drain carries one
    # wait per pending engine clock and fails to compile. Split the waits
    # across single-wait SP nops; SP executes in program order, so the
    # drain still orders after everything.
    nc = self.nc
    probe = nc.sync.nop().ins
    wait_clock.add_sem_waits(probe, ScopedClock({None: tick_clock.global_clock}))
    waits = list(probe.sync_info.on_wait) if probe.sync_info else []
    probe.sync_info = mybir.SyncInfo(on_wait=waits[:1], on_update=[])
    for w in waits[1:]:
        extra = nc.sync.nop().ins
        extra.sync_info = mybir.SyncInfo(on_wait=[w], on_update=[])
    nc.sync.drain()
    nc.all_engine_barrier()
    assert self.sems is not None
    popped = nc._tile_sem_poison_stack.pop()
    assert popped is self._sem_poison
    nc.clear_and_free_semaphores(list(self.sems.allocated().values()))
    nc.all_engine_barrier()


tile.TileContext._drain_and_barrier = _patched_drain_and_barrier


def _split_excess_waits(nc: bass.Bass, max_waits: int = 1):
    # Same walrus limitation, general form: cap sync waits per instruction,
    # hoisting the excess onto same-engine NOPs inserted just before (the
    # engine executes its stream in order, so semantics are unchanged).
    for bb in nc.main_func.blocks:
        insts = list(bb.instructions)
        out, changed = [], False
        for ins in insts:
            si = ins.sync_info
            waits = list(si.on_wait) if (si is not None and si.on_wait) else []
            if len(waits) > max_waits:
                ups = list(si.on_update) if si.on_update else []
                for w in waits[:-max_waits]:
                    nop = mybir.InstNoOp(
                        name=nc.get_next_instruction_name(), ins=[], outs=[])
                    nop.engine = ins.engine
                    nop.sync_info = mybir.SyncInfo(on_wait=[w], on_update=[])
                    nc.register_instruction(nop)
                    out.append(nop)
                ins.sync_info = mybir.SyncInfo(
                    on_wait=waits[-max_waits:], on_update=ups)
                changed = True
            out.append(ins)
        if changed:
            bb.instructions = out


def build_nc(bench_iters: int = 0) -> bass.Bass:
    # bench_iters>0 wraps the compute body in a HW For_i loop so one
    # dispatch runs it N times (timing two N values cancels dispatch cost).
    nc = bass.Bass()
    x_dram = nc.declare_dram_parameter("pred_logit", [ROWS, C], f32, isOutput=False)
    gt_dram = nc.declare_dram_parameter("gt", [ROWS, C], f32, isOutput=False)
    out_dram = nc.declare_dram_parameter("partials", [N_SLICE, 512], f32, isOutput=True)
    ce_dram = nc.declare_dram_parameter("ce_part", [1, 512], f32, isOutput=True)

    from contextlib import ExitStack
    with tile.TileContext(nc) as tc, ExitStack() as es:
        consts = es.enter_context(tc.tile_pool(name="consts", bufs=1))
        xpool = es.enter_context(tc.tile_pool(name="xpool", bufs=2))
        tpool = es.enter_context(tc.tile_pool(name="tpool", bufs=2))
        rowp = es.enter_context(tc.tile_pool(name="rowp", bufs=2))
        ck = es.enter_context(tc.tile_pool(name="ck", bufs=2))
        psum = es.enter_context(tc.tile_pool(name="psum", bufs=1, space="PSUM"))

        ones = consts.tile([128, 1], bf16)
        nc.vector.memset(ones, 1.0)
        neg_ones = consts.tile([128, 1], bf16)
        nc.vector.memset(neg_ones, -1.0)
        eps_half = consts.tile([128, 1], f32)
        nc.vector.memset(eps_half, 0.5 * EPS * _EK)

        # Column-sum accumulators for contrib = e1+e2-e3: one [1,512] row
        # per 512-column slice. PE output base partition must be 0/32/64,
        # so pack 3 slices per PSUM bank at those bases.
        banks = [psum.tile([128, 512], f32, name=f"csbank{i}", tag=f"csbank{i}")
                 for i in range((N_SLICE + 2) // 3)]
        def cs_ap(m):
            bank, base = banks[m // 3], 32 * (m % 3)
            return bank[base:base + 1, :]
        # CE accumulator: column sums of e3 from ALL slices superimposed
        # into one 512-wide row; the host sums the 512 values.
        ce_psum = psum.tile([1, 512], f32)

        N_XSUB = 4
        XS = C // N_XSUB

        def emit_body():
          for b in range(N_BLK):
            r0 = b * 128
            # split the x load + exp pass into sub-chunks so ACT can start
            # as soon as the first piece lands (instead of stalling ~12us
            # behind one monolithic 4MB DMA)
            xb = xpool.tile([128, C], f32, tag="x")
            tb = tpool.tile([128, C], bf16, tag="t")
            s4 = rowp.tile([128, N_XSUB], f32, tag="s4")
            for ix in range(N_XSUB):
                xsl = slice(ix * XS, (ix + 1) * XS)
                nc.sync.dma_start(out=xb[:, xsl], in_=x_dram[r0:r0 + 128, xsl])
                nc.scalar.activation(
                    out=tb[:, xsl], in_=xb[:, xsl], func=AF.Exp,
                    accum_out=s4[:, ix:ix + 1],
                )
            s = rowp.tile([128, 1], f32, tag="s")
            nc.vector.tensor_reduce(
                out=s[:], in_=s4[:], op=ALU.add, axis=mybir.AxisListType.X,
            )

            recip = rowp.tile([128, 1], f32, tag="recip")
            nc.vector.reciprocal(out=recip[:], in_=s[:])
            # lse - K  (so xp = x - lse + K comes out of one tensor_scalar)
            lse = rowp.tile([128, 1], f32, tag="lse")
            nc.scalar.activation(out=lse[:], in_=s[:], func=AF.Ln, scale=_EMK)
            neg_lse = rowp.tile([128, 1], f32, tag="neg_lse")
            if XP_ON_ACT:
                nc.scalar.mul(neg_lse[:], lse[:], -1.0)

            for c in range(N_CHUNK):
                j0 = c * F2
                gtc = ck.tile([128, F2], f32, tag="gt")
                nc.sync.dma_start(out=gtc[:], in_=gt_dram[r0:r0 + 128, j0:j0 + F2])

                gt16 = ck.tile([128, F2], bf16, tag="gt16")
                if GT16_ON_ACT:
                    nc.scalar.copy(out=gt16[:], in_=gtc[:])
                else:
                    nc.vector.tensor_copy(out=gt16[:], in_=gtc[:])
                lngt = ck.tile([128, F2], bf16, tag="lngt")
                nc.scalar.activation(out=lngt[:], in_=gtc[:], func=AF.Ln, scale=_EK)

                xp = ck.tile([128, F2], bf16, tag="xp")
                if XP_ON_ACT:
                    nc.scalar.activation(
                        out=xp[:], in_=xb[:, j0:j0 + F2], func=AF.Identity,
                        bias=neg_lse[:],
                    )
                else:
                    nc.vector.tensor_scalar(
                        out=xp[:], in0=xb[:, j0:j0 + F2],
                        scalar1=lse[:], scalar2=None, op0=ALU.subtract,
                    )
                # u = t*recip + gt = p + gt
                u = ck.tile([128, F2], bf16, tag="u")
                if USE_STT:
                    nc.vector.scalar_tensor_tensor(
                        out=u[:], in0=tb[:, j0:j0 + F2], scalar=recip[:],
                        in1=gt16[:], op0=ALU.mult, op1=ALU.add,
                    )
                else:
                    p = ck.tile([128, F2], bf16, tag="p", bufs=1)
                    nc.vector.tensor_scalar(
                        out=p[:], in0=tb[:, j0:j0 + F2],
                        scalar1=recip[:], scalar2=None, op0=ALU.mult,
                    )
                    nc.vector.tensor_tensor(out=u[:], in0=gt16[:], in1=p[:], op=ALU.add)
                logm = ck.tile([128, F2], bf16, tag="logm")
                nc.scalar.activation(
                    out=logm[:], in_=u[:], func=AF.Ln, scale=0.5 * _EK,
                    bias=eps_half[:],
                )
                q = ck.tile([128, F2], bf16, tag="q")
                nc.vector.tensor_tensor(out=q[:], in0=xp[:], in1=logm[:], op=ALU.subtract)
                r = ck.tile([128, F2], bf16, tag="r")
                nc.vector.tensor_tensor(out=r[:], in0=lngt[:], in1=logm[:], op=ALU.subtract)

                # contrib = gt*(lngt-logm) + p*(xp-logm)  (shifts cancel)
                e1 = ck.tile([128, F2], bf16, tag="e1", bufs=1)
                nc.vector.tensor_tensor(out=e1[:], in0=gt16[:], in1=r[:], op=ALU.mult)
                # e2 = (t*recip)*q = p*q
                e2 = ck.tile([128, F2], bf16, tag="e2", bufs=1)
                if USE_STT:
                    nc.vector.scalar_tensor_tensor(
                        out=e2[:], in0=tb[:, j0:j0 + F2], scalar=recip[:],
                        in1=q[:], op0=ALU.mult, op1=ALU.mult,
                    )
                else:
                    nc.vector.tensor_tensor(out=e2[:], in0=p[:], in1=q[:], op=ALU.mult)
                e3 = ck.tile([128, F2], bf16, tag="e3", bufs=1)
                nc.vector.tensor_tensor(out=e3[:], in0=gt16[:], in1=xp[:], op=ALU.mult)

                for k in range(F2 // 512):
                    m = (j0 + k * 512) // 512
                    sl = slice(k * 512, (k + 1) * 512)
                    nc.tensor.matmul(
                        cs_ap(m), ones[:], e1[:, sl],
                        start=(b == 0), stop=False,
                    )
                    nc.tensor.matmul(
                        cs_ap(m), ones[:], e2[:, sl],
                        start=False, stop=(b == N_BLK - 1),
                    )
                    nc.tensor.matmul(
                        ce_psum[:], ones[:], e3[:, sl],
                        start=(b == 0 and m == 0),
                        stop=(b == N_BLK - 1 and m == N_SLICE - 1),
                    )

        if bench_iters > 1:
            with tc.For_i(0, bench_iters, 1):
                emit_body()
        else:
            emit_body()

        # PSUM is not DMA-readable: bounce through SBUF via ScalarE.
        sb_banks = [consts.tile([128, 512], f32, name=f"sb_cs{i}", tag=f"sb_cs{i}")
                    for i in range(len(banks))]
        for i, bank in enumerate(banks):
            nc.scalar.copy(out=sb_banks[i][:], in_=bank[:])
        sb_ce = consts.tile([1, 512], f32)
        nc.scalar.copy(out=sb_ce[:], in_=ce_psum[:])
        for m in range(N_SLICE):
            bank, base = sb_banks[m // 3], 32 * (m % 3)
            nc.sync.dma_start(out=out_dram[m:m + 1, :], in_=bank[base:base + 1, :])
        nc.sync.dma_start(out=ce_dram[:], in_=sb_ce[:])

    _split_excess_waits(nc)
    return nc


_NC_CACHE = None
LAST_EXEC_NS = None
LAST_TRACE = None
LAST_PROFILE_JSON = None


def make_in_maps(pred_logit: np.ndarray, gt: np.ndarray) -> list[dict]:
    pred_logit = np.ascontiguousarray(pred_logit, dtype=np.float32)
    gt = np.ascontiguousarray(gt, dtype=np.float32)
    return [
        {
            "pred_logit": pred_logit[c * ROWS:(c + 1) * ROWS],
            "gt": gt[c * ROWS:(c + 1) * ROWS],
        }
        for c in range(N_CORES)
    ]


def kernel(pred_logit: np.ndarray, gt: np.ndarray) -> np.ndarray:
    global _NC_CACHE, LAST_EXEC_NS, LAST_TRACE, LAST_PROFILE_JSON
    if _NC_CACHE is None:
        _NC_CACHE = build_nc()
    nc = _NC_CACHE

    in_maps = make_in_maps(pred_logit, gt)
    res = run_bass_kernel_spmd(nc, in_maps, list(range(N_CORES)))
    if res.exec_time_ns is not None:
        LAST_EXEC_NS = res.exec_time_ns
        LAST_TRACE = res.instructions_and_trace
        LAST_PROFILE_JSON = res.profile_json

    w = (C - np.arange(C)).astype(np.float64)
    e1_total = 0.0   # sum_ij w_j * contrib
    ce_total = 0.0   # sum_ij gt * logp
    for r in res.results:
        cs = r["partials"].astype(np.float64).reshape(C)
        e1_total += np.dot(w, cs)
        # ce plane accumulated gt*(x - lse + K); remove the K shift
        # (rows of gt sum to 1, so sum_ij K*gt = K*ROWS per core)
        ce_total += float(r["ce_part"].astype(np.float64).sum()) - K_LSE * ROWS
    loss = -ce_total / B + 0.25 * e1_total / B
    return np.array(loss, dtype=np.float32)

